# revision 32
# baseline (speedup 1.0000x reference)
"""BitPackedLinear Trainium2 kernel (8-core SPMD, token-sharded, fp8 DoubleRow).

y = x @ W.T + bias, W = unpack_bits(packed_weight) in {-1,+1}, shapes:
  x [2, 2048, 4096] f32, packed_weight [4096, 512] u8, bias [4096] f32.

Sharding: data-parallel over tokens (4096 tokens -> 512/core). Each core
computes y_c = x_c @ W.T + bias for its token shard against the full
weight; the host just concatenates shards.

Device algorithm per core:
  - W = 2B - 1, so y = 2*(x@B.T) - rowsum(x) + bias. The matmul runs on
    B2 = 2B in {0,2} (exact in fp8 e4m3, pattern 0x40).
  - x is split hi/lo straight from f32: xh = e4m3(x), xl = e4m3(x - xh).
    The pair carries ~14 mantissa bits, so fp8 DoubleRow matmuls (both
    operands e4m3, 2 k-rows per PE cell, 0.5 cyc/row) beat the bf16
    matmul at half the PE time with comparable accuracy.
  - Pairing avoids duplicating weights: MM j contracts (xh_j, xl_{j+1})
    against (w_j, w_{j+1}); summed over j=0..31 (with wraparound copies
    w_32=w_0, xl-slot 65=xl_0) this equals sum_j w_j*(xh_j + xl_j).
    xT8 is interleaved [hi_0, lo_0, hi_1, lo_1, ...]; the lhsT pair is a
    stride-3 stepped slice; the rhs pair is two adjacent wt slots.
  - Contraction (i) is tiled bit-sliced: i-tile j=(kt,b) = {8*(128*kt+k')+b},
    so every weight tile is one shift+mask from transposed packed bytes.
  - x rides fast HWDGE DMAs as f32 and is PE-transposed as f32 (2 cyc/row,
    4 token-tiles batched per PSUM bank); ACT casts hi out of PSUM, DVE
    subtracts the residual. No bf16 bounce anywhere.
  - byteT_u8[k',kt,o] = pw[o,128*kt+k'] via PE pass-through transposes of
    gpsimd-cast bf16 bytes (ACT moves them out); adjacent o bytes viewed
    as u16 lanes let one tensor_scalar (shl 6-b & 0x4040, shr 1 for b=7)
    unpack TWO fp8 weights per lane at the DVE 4x rate. The whole weight
    chain for slab sl+1 is emitted before slab sl's matmuls (one-slab
    prefetch through bufs=2 pools).
  - Matmuls are emitted j-major for slab 0 (streaming against the hi/lo
    and unpack producers), t-major afterwards (groups finish staggered so
    the epilogue frees PSUM banks before the slab boundary).
  - s_col[t] = -sum_i(xh+xl) via a (-1)-ones DoubleRow matmul chasing the
    same pairs; its [1,T] psum row is PE-transposed back to [t,1] layout.
  - bias is broadcast to all partitions by a stride-0 DMA; the DVE
    epilogue fuses (psum + neg_s) + bias into the PSUM->SBUF move.
"""
import sys

sys.path.insert(0, "/opt/trn_rl_repo")
from contextlib import ExitStack

import numpy as np

import concourse.tile as tile
from concourse import bacc, mybir
from concourse.bass import ts
from concourse.bass_utils import run_bass_kernel_spmd
from concourse.masks import make_identity

F32 = mybir.dt.float32
F32R = mybir.dt.float32r
BF16 = mybir.dt.bfloat16
U8 = mybir.dt.uint8
U16 = mybir.dt.uint16
FP8 = mybir.dt.float8e4
P = 128

N_CORES = 8
B_DIM, S_DIM, I_DIM, O_DIM = 2, 2048, 4096, 4096
T_FULL = B_DIM * S_DIM          # 4096 tokens
T_SHARD = T_FULL // N_CORES     # 512 tokens per core
OUT_NAME = "y"
OUT_SHAPE = (T_SHARD, O_DIM)

DR = mybir.MatmulPerfMode.DoubleRow
SHL = mybir.AluOpType.logical_shift_left
SHR = mybir.AluOpType.logical_shift_right
AND = mybir.AluOpType.bitwise_and
SUB = mybir.AluOpType.subtract
ADD = mybir.AluOpType.add


def build(T=T_SHARD, I=I_DIM, O=O_DIM, O_SLAB=512, n_cores=N_CORES):
    assert I % 1024 == 0 and T % P == 0 and O % P == 0 and O % O_SLAB == 0
    KT = I // 1024          # 128-byte groups along i
    NJ = KT * 8             # bit-sliced i-tiles (j = kt*8 + b)
    TT = T // P             # token tiles
    K = I // 8              # packed bytes per weight row
    NSLAB = O // O_SLAB
    OSL_T = O_SLAB // P
    OSL2 = O_SLAB // 2      # u16 pair lanes per slab

    nc = bacc.Bacc("TRN2", target_bir_lowering=False, debug=False,
                   num_devices=n_cores)
    x_d = nc.dram_tensor("x", [T, I], F32, kind="ExternalInput").ap()
    pw_d = nc.dram_tensor("pw", [O, K], U8, kind="ExternalInput").ap()
    bias_d = nc.dram_tensor("bias", [O], F32, kind="ExternalInput").ap()
    y_d = nc.dram_tensor(OUT_NAME, [T, O], F32, kind="ExternalOutput").ap()

    with tile.TileContext(nc) as tc:
        with ExitStack() as ctx:
            const = ctx.enter_context(tc.tile_pool(name="const", bufs=1))
            persist = ctx.enter_context(tc.tile_pool(name="persist", bufs=1))

            ident_bf = const.tile([P, P], BF16)
            make_identity(nc, ident_bf[:])
            ident_f = const.tile([P, P], F32)
            make_identity(nc, ident_f[:])
            nones8 = const.tile([P, 2, P], FP8)
            nc.vector.memset(nones8[:], -1.0)

            xT8 = persist.tile([P, 2 * NJ + 2, T], FP8)  # hi/lo interleaved
            srow = persist.tile([1, T], F32)
            neg_s = persist.tile([P, TT], F32)
            pw_ap = pw_d.rearrange("(ot p) k -> p ot k", p=P)
            bias_bc = bias_d.rearrange("(b o) -> b o", b=1)

            pk_pool = ctx.enter_context(tc.tile_pool(name="pk", bufs=2))
            byteT_pool = ctx.enter_context(tc.tile_pool(name="byteT", bufs=2))
            pkbf_pool = ctx.enter_context(tc.tile_pool(name="pkbf", bufs=2))
            x32_pool = ctx.enter_context(tc.tile_pool(name="x32", bufs=4))
            xnat_pool = ctx.enter_context(
                tc.tile_pool(name="xnat", bufs=2 * KT * TT)
            )
            wt_pool = ctx.enter_context(tc.tile_pool(name="wt", bufs=2))
            bbc_pool = ctx.enter_context(tc.tile_pool(name="bbc", bufs=2))
            y_pool = ctx.enter_context(tc.tile_pool(name="ysb", bufs=3))
            ps_tr = ctx.enter_context(
                tc.tile_pool(name="ps_tr", bufs=2, space="PSUM")
            )
            ps_trb = ctx.enter_context(
                tc.tile_pool(name="ps_trb", bufs=1, space="PSUM")
            )
            ps_mm = ctx.enter_context(
                tc.tile_pool(name="ps_mm", bufs=4, space="PSUM")
            )
            ps_s_pool = ctx.enter_context(
                tc.tile_pool(name="ps_s", bufs=1, space="PSUM")
            )

            def byte_dma(sl):
                pk = pk_pool.tile([P, OSL_T, K], U8)
                nc.sync.dma_start(pk[:], pw_ap[:, ts(sl, OSL_T), :])
                return pk

            def byte_slab(pk):
                """Transposed packed bytes for one o-slab, u8 [k', kt, o];
                adjacent-o pairs are read back as u16 lanes by the unpack."""
                byteT = byteT_pool.tile([P, KT, O_SLAB], U8)
                for otl in range(OSL_T):
                    pkbf = pkbf_pool.tile([P, K], BF16)
                    nc.gpsimd.tensor_copy(out=pkbf[:], in_=pk[:, otl, :])
                    ps = ps_trb.tile([P, KT, P], BF16, tag="trb_ps")
                    for kt in range(KT):
                        nc.tensor.transpose(
                            ps[:, kt, :], pkbf[:, ts(kt, P)], ident_bf[:]
                        )
                    # one strided copy back: [k', kt, o-block of 128]
                    nc.scalar.copy(out=byteT[:, :, ts(otl, P)], in_=ps[:])
                return byteT

            def unpack_slab(byteT):
                """wt slots 0..NJ-1 = weight i-tiles, slot NJ = copy of 0."""
                wt = wt_pool.tile([P, NJ + 1, OSL2], U16)
                for slot in range(NJ + 1):
                    j = slot % NJ
                    kt, b = divmod(j, 8)
                    src = byteT[:, kt, :].bitcast(U16)
                    sh, op = (6 - b, SHL) if b < 7 else (1, SHR)
                    nc.vector.tensor_scalar(
                        out=wt[:, slot, :], in0=src, scalar1=sh,
                        scalar2=0x4040, op0=op, op1=AND,
                    )
                return wt

            # slab 0 bytes + all x chunks (fast HWDGE, raw f32) up front
            pk0 = byte_dma(0)
            x32s = {}
            for kt in range(KT):
                for tt in range(TT):
                    x32 = x32_pool.tile([P, P, 8], F32)
                    nc.sync.dma_start(
                        x32[:],
                        x_d[ts(tt, P), ts(kt, 1024)].rearrange(
                            "p (k b) -> p k b", b=8
                        ),
                    )
                    x32s[kt, tt] = x32
            byteT0 = byte_slab(pk0)

            # bf16 casts split across engines so no single queue gates the
            # transpose stream (kt0 -> DVE, kt1 -> ACT, kt2/3 -> gpsimd)
            xns = {}
            for kt in range(KT):
                for tt in range(TT):
                    xn = xnat_pool.tile([P, P, 8], BF16, tag="xn16")
                    if kt == 0:
                        nc.vector.tensor_copy(out=xn[:], in_=x32s[kt, tt][:])
                    elif kt == 1:
                        nc.scalar.copy(out=xn[:], in_=x32s[kt, tt][:])
                    else:
                        nc.gpsimd.tensor_copy(out=xn[:], in_=x32s[kt, tt][:])
                    xns[kt, tt] = xn

            wt_cur = unpack_slab(byteT0)

            def emit_xpose(j):
                # bf16 PE transposes (4 token-tiles per PSUM bank); hi
                # (ACT, e4m3 cast out of PSUM) and lo (DVE residual)
                kt, b = divmod(j, 8)
                ps = ps_tr.tile([P, TT, P], BF16, tag="trx_ps")
                for tt in range(TT):
                    nc.tensor.transpose(
                        ps[:, tt, :], xns[kt, tt][:, :, b], ident_bf[:]
                    )
                nc.scalar.copy(out=xT8[:, 2 * j, :], in_=ps[:])
                nc.vector.tensor_tensor(
                    out=xT8[:, 2 * j + 1, :],
                    in0=ps[:].rearrange("p a b -> p (a b)"),
                    in1=xT8[:, 2 * j, :], op=SUB,
                )
                if j == 0:
                    # wraparound lo_0 copy (slot 2NJ is a never-read pad)
                    nc.vector.tensor_copy(
                        out=xT8[:, 2 * NJ + 1, :], in_=xT8[:, 1, :]
                    )

            def pair_ap(j, t0, tn):
                # (hi_j, lo_{j+1}): slots 2j, 2j+3 -> stride-3 stepped slice
                return xT8[:, 2 * j:2 * j + 4:3, t0:t0 + tn]

            s_ps = ps_s_pool.tile([P, T], F32)

            # main o-slab loop; weights for slab sl+1 are emitted before
            # slab sl's matmuls (one-slab prefetch)
            def epilogue(sl, tsub, ps_y, bbc):
                y_sb = y_pool.tile([P, O_SLAB], F32)
                nc.vector.scalar_tensor_tensor(
                    out=y_sb[:], in0=ps_y[:],
                    scalar=neg_s[:, tsub:tsub + 1],
                    in1=bbc[:], op0=ADD, op1=ADD,
                )
                nc.sync.dma_start(
                    y_d[ts(tsub, P), ts(sl, O_SLAB)], y_sb[:]
                )

            for sl in range(NSLAB):
                wt = wt_cur
                pk_next = byte_dma(sl + 1) if sl + 1 < NSLAB else None
                wt8 = wt[:].bitcast(FP8)  # [P, NJ+1, O_SLAB]

                bbc = bbc_pool.tile([P, O_SLAB], F32)
                nc.sync.dma_start(
                    bbc[:],
                    bias_bc[:, ts(sl, O_SLAB)].partition_broadcast(P),
                )
                pss = [
                    ps_mm.tile([P, O_SLAB], F32, name=f"ps{t}", tag="ps_y")
                    for t in range(TT)
                ]
                if sl == 0:
                    # j-major, transposes interleaved 2 tiles ahead: the
                    # in-order PE reaches each transpose just before the
                    # matmuls that consume its hi/lo; the neg-rowsum MMs
                    # ride the same stream
                    STAG = 2
                    for j in range(STAG):
                        emit_xpose(j)
                    for j in range(NJ):
                        if j + STAG < NJ:
                            emit_xpose(j + STAG)
                        nc.tensor.matmul(
                            s_ps[:], nones8[:], pair_ap(j, 0, T),
                            start=(j == 0), stop=(j == NJ - 1),
                            perf_mode=DR,
                        )
                        for tsub in range(TT):
                            nc.tensor.matmul(
                                pss[tsub][:],
                                pair_ap(j, tsub * P, P),
                                wt8[:, j:j + 2, :],
                                start=(j == 0), stop=(j == NJ - 1),
                                perf_mode=DR,
                            )
                    # transpose the [1,T] neg-rowsum back to [t,1] layout
                    nc.vector.tensor_copy(out=srow[:], in_=s_ps[0:1, :])
                    ps_s2 = ps_tr.tile([P, TT, P], F32, tag="trx_ps")
                    for c in range(TT):
                        nc.tensor.transpose(
                            ps_s2[:, c, 0:1], srow[0:1, ts(c, P)],
                            ident_f[0:1, 0:1],
                        )
                    nc.vector.tensor_copy(
                        out=neg_s[:], in_=ps_s2[:, :, 0]
                    )
                    for tsub in range(TT):
                        epilogue(sl, tsub, pss[tsub], bbc)
                    if pk_next is not None:
                        wt_cur = unpack_slab(byte_slab(pk_next))
                else:
                    # t-major: groups finish staggered; each group's
                    # epilogue follows its matmuls so PSUM banks free
                    # early. The next slab's weight compute is emitted
                    # mid-slab so the PE reaches those transposes only
                    # after their inputs are ready (in-order engine).
                    for tsub in range(TT):
                        for j in range(NJ):
                            nc.tensor.matmul(
                                pss[tsub][:],
                                pair_ap(j, tsub * P, P),
                                wt8[:, j:j + 2, :],
                                start=(j == 0), stop=(j == NJ - 1),
                                perf_mode=DR,
                            )
                        epilogue(sl, tsub, pss[tsub], bbc)
                        if tsub == 1 and pk_next is not None:
                            wt_cur = unpack_slab(byte_slab(pk_next))

    nc.compile()
    return nc


_NC = None


def _get_nc():
    global _NC
    if _NC is None:
        _NC = build()
    return _NC


def run(x, packed_weight, bias, trace=False):
    x = np.ascontiguousarray(np.asarray(x, dtype=np.float32))
    pw = np.ascontiguousarray(np.asarray(packed_weight, dtype=np.uint8))
    bias = np.ascontiguousarray(np.asarray(bias, dtype=np.float32))
    assert x.shape == (B_DIM, S_DIM, I_DIM)
    assert pw.shape == (O_DIM, I_DIM // 8)
    assert bias.shape == (O_DIM,)

    nc = _get_nc()
    xs = x.reshape(T_FULL, I_DIM)
    in_maps = [
        {
            "x": np.ascontiguousarray(xs[c * T_SHARD:(c + 1) * T_SHARD]),
            "pw": pw,
            "bias": bias,
        }
        for c in range(N_CORES)
    ]
    res = run_bass_kernel_spmd(nc, in_maps, list(range(N_CORES)), trace=trace)
    y = np.concatenate(
        [res.results[c][OUT_NAME] for c in range(N_CORES)], axis=0
    )
    return y.reshape(B_DIM, S_DIM, O_DIM), res


def kernel(x, packed_weight, bias):
    y, _ = run(x, packed_weight, bias, trace=False)
    return y


# revision 33
# speedup vs baseline: 1.0428x; 1.0428x over previous
"""BitPackedLinear Trainium2 kernel (8-core SPMD, token-sharded, fp8 DoubleRow).

y = x @ W.T + bias, W = unpack_bits(packed_weight) in {-1,+1}, shapes:
  x [2, 2048, 4096] f32, packed_weight [4096, 512] u8, bias [4096] f32.

Sharding: data-parallel over tokens (4096 tokens -> 512/core). Each core
computes y_c = x_c @ W.T + bias for its token shard against the full
weight; the host just concatenates shards.

Device algorithm per core:
  - W = 2B - 1, so y = 2*(x@B.T) - rowsum(x) + bias. The matmul runs on
    B2 = 2B in {0,2} (exact in fp8 e4m3, pattern 0x40).
  - x is split hi/lo straight from f32: xh = e4m3(x), xl = e4m3(x - xh).
    The pair carries ~14 mantissa bits, so fp8 DoubleRow matmuls (both
    operands e4m3, 2 k-rows per PE cell, 0.5 cyc/row) beat the bf16
    matmul at half the PE time with comparable accuracy.
  - Pairing avoids duplicating weights: MM j contracts (xh_j, xl_{j+1})
    against (w_j, w_{j+1}); summed over j=0..31 (with wraparound copies
    w_32=w_0, xl-slot 65=xl_0) this equals sum_j w_j*(xh_j + xl_j).
    xT8 is interleaved [hi_0, lo_0, hi_1, lo_1, ...]; the lhsT pair is a
    stride-3 stepped slice; the rhs pair is two adjacent wt slots.
  - Contraction (i) is tiled bit-sliced: i-tile j=(kt,b) = {8*(128*kt+k')+b},
    so every weight tile is one shift+mask from transposed packed bytes.
  - x rides fast HWDGE DMAs as f32 and is PE-transposed as f32 (2 cyc/row,
    4 token-tiles batched per PSUM bank); ACT casts hi out of PSUM, DVE
    subtracts the residual. No bf16 bounce anywhere.
  - byteT_u8[k',kt,o] = pw[o,128*kt+k'] via PE pass-through transposes of
    gpsimd-cast bf16 bytes (ACT moves them out); adjacent o bytes viewed
    as u16 lanes let one tensor_scalar (shl 6-b & 0x4040, shr 1 for b=7)
    unpack TWO fp8 weights per lane at the DVE 4x rate. The whole weight
    chain for slab sl+1 is emitted before slab sl's matmuls (one-slab
    prefetch through bufs=2 pools).
  - Matmuls are emitted j-major for slab 0 (streaming against the hi/lo
    and unpack producers), t-major afterwards (groups finish staggered so
    the epilogue frees PSUM banks before the slab boundary).
  - s_col[t] = -sum_i(xh+xl) via a (-1)-ones DoubleRow matmul chasing the
    same pairs; its [1,T] psum row is PE-transposed back to [t,1] layout.
  - bias is broadcast to all partitions by a stride-0 DMA; the DVE
    epilogue fuses (psum + neg_s) + bias into the PSUM->SBUF move.
"""
import sys

sys.path.insert(0, "/opt/trn_rl_repo")
from contextlib import ExitStack

import numpy as np

import concourse.tile as tile
from concourse import bacc, mybir
from concourse.bass import ts
from concourse.bass_utils import run_bass_kernel_spmd
from concourse.masks import make_identity

F32 = mybir.dt.float32
F32R = mybir.dt.float32r
BF16 = mybir.dt.bfloat16
U8 = mybir.dt.uint8
U16 = mybir.dt.uint16
FP8 = mybir.dt.float8e4
P = 128

N_CORES = 8
B_DIM, S_DIM, I_DIM, O_DIM = 2, 2048, 4096, 4096
T_FULL = B_DIM * S_DIM          # 4096 tokens
T_SHARD = T_FULL // N_CORES     # 512 tokens per core
OUT_NAME = "y"
OUT_SHAPE = (T_SHARD, O_DIM)

DR = mybir.MatmulPerfMode.DoubleRow
SHL = mybir.AluOpType.logical_shift_left
SHR = mybir.AluOpType.logical_shift_right
AND = mybir.AluOpType.bitwise_and
SUB = mybir.AluOpType.subtract
ADD = mybir.AluOpType.add


def build(T=T_SHARD, I=I_DIM, O=O_DIM, O_SLAB=512, n_cores=N_CORES):
    assert I % 1024 == 0 and T % P == 0 and O % P == 0 and O % O_SLAB == 0
    KT = I // 1024          # 128-byte groups along i
    NJ = KT * 8             # bit-sliced i-tiles (j = kt*8 + b)
    TT = T // P             # token tiles
    K = I // 8              # packed bytes per weight row
    NSLAB = O // O_SLAB
    OSL_T = O_SLAB // P
    OSL2 = O_SLAB // 2      # u16 pair lanes per slab

    nc = bacc.Bacc("TRN2", target_bir_lowering=False, debug=False,
                   num_devices=n_cores)
    x_d = nc.dram_tensor("x", [T, I], F32, kind="ExternalInput").ap()
    pw_d = nc.dram_tensor("pw", [O, K], U8, kind="ExternalInput").ap()
    bias_d = nc.dram_tensor("bias", [O], F32, kind="ExternalInput").ap()
    y_d = nc.dram_tensor(OUT_NAME, [T, O], F32, kind="ExternalOutput").ap()

    with tile.TileContext(nc) as tc:
        with ExitStack() as ctx:
            const = ctx.enter_context(tc.tile_pool(name="const", bufs=1))
            persist = ctx.enter_context(tc.tile_pool(name="persist", bufs=1))

            ident_bf = const.tile([P, P], BF16)
            make_identity(nc, ident_bf[:])
            ident_f = const.tile([P, P], F32)
            make_identity(nc, ident_f[:])
            nones8 = const.tile([P, 2, P], FP8)
            nc.vector.memset(nones8[:], -1.0)

            xT8 = persist.tile([P, 2 * NJ + 2, T], FP8)  # hi/lo interleaved
            srow = persist.tile([1, T], F32)
            neg_s = persist.tile([P, TT], F32)
            pw_ap = pw_d.rearrange("(ot p) k -> p ot k", p=P)
            bias_bc = bias_d.rearrange("(b o) -> b o", b=1)

            pk_pool = ctx.enter_context(tc.tile_pool(name="pk", bufs=2))
            byteT_pool = ctx.enter_context(tc.tile_pool(name="byteT", bufs=2))
            pkbf_pool = ctx.enter_context(tc.tile_pool(name="pkbf", bufs=2))
            x32_pool = ctx.enter_context(tc.tile_pool(name="x32", bufs=4))
            xnat_pool = ctx.enter_context(
                tc.tile_pool(name="xnat", bufs=2 * KT * TT)
            )
            wt_pool = ctx.enter_context(tc.tile_pool(name="wt", bufs=2))
            bbc_pool = ctx.enter_context(tc.tile_pool(name="bbc", bufs=2))
            y_pool = ctx.enter_context(tc.tile_pool(name="ysb", bufs=3))
            ps_tr = ctx.enter_context(
                tc.tile_pool(name="ps_tr", bufs=2, space="PSUM")
            )
            ps_trb = ctx.enter_context(
                tc.tile_pool(name="ps_trb", bufs=1, space="PSUM")
            )
            ps_mm = ctx.enter_context(
                tc.tile_pool(name="ps_mm", bufs=4, space="PSUM")
            )
            ps_s_pool = ctx.enter_context(
                tc.tile_pool(name="ps_s", bufs=1, space="PSUM")
            )

            def byte_dma(sl):
                pk = pk_pool.tile([P, OSL_T, K], U8)
                nc.sync.dma_start(pk[:], pw_ap[:, ts(sl, OSL_T), :])
                return pk

            def byte_slab(pk):
                """Transposed packed bytes for one o-slab, u8 [k', kt, o];
                adjacent-o pairs are read back as u16 lanes by the unpack."""
                byteT = byteT_pool.tile([P, KT, O_SLAB], U8)
                for otl in range(OSL_T):
                    pkbf = pkbf_pool.tile([P, K], BF16)
                    nc.gpsimd.tensor_copy(out=pkbf[:], in_=pk[:, otl, :])
                    ps = ps_trb.tile([P, KT, P], BF16, tag="trb_ps")
                    for kt in range(KT):
                        nc.tensor.transpose(
                            ps[:, kt, :], pkbf[:, ts(kt, P)], ident_bf[:]
                        )
                    # one strided copy back: [k', kt, o-block of 128]
                    nc.scalar.copy(out=byteT[:, :, ts(otl, P)], in_=ps[:])
                return byteT

            def unpack_slab(byteT):
                """wt slots 0..NJ-1 = weight i-tiles, slot NJ = copy of 0."""
                wt = wt_pool.tile([P, NJ + 1, OSL2], U16)
                for slot in range(NJ + 1):
                    j = slot % NJ
                    kt, b = divmod(j, 8)
                    src = byteT[:, kt, :].bitcast(U16)
                    sh, op = (6 - b, SHL) if b < 7 else (1, SHR)
                    nc.vector.tensor_scalar(
                        out=wt[:, slot, :], in0=src, scalar1=sh,
                        scalar2=0x4040, op0=op, op1=AND,
                    )
                return wt

            # slab 0 bytes + all x chunks (fast HWDGE, raw f32) up front
            pk0 = byte_dma(0)
            x32s = {}
            for kt in range(KT):
                for tt in range(TT):
                    x32 = x32_pool.tile([P, P, 8], F32)
                    nc.sync.dma_start(
                        x32[:],
                        x_d[ts(tt, P), ts(kt, 1024)].rearrange(
                            "p (k b) -> p k b", b=8
                        ),
                    )
                    x32s[kt, tt] = x32
            byteT0 = byte_slab(pk0)

            # bf16 casts split across engines so no single queue gates the
            # transpose stream (kt0 -> DVE, kt1 -> ACT, kt2/3 -> gpsimd)
            xns = {}
            for kt in range(KT):
                for tt in range(TT):
                    xn = xnat_pool.tile([P, P, 8], BF16, tag="xn16")
                    if kt == 0:
                        nc.vector.tensor_copy(out=xn[:], in_=x32s[kt, tt][:])
                    elif kt == 1:
                        nc.scalar.copy(out=xn[:], in_=x32s[kt, tt][:])
                    else:
                        nc.gpsimd.tensor_copy(out=xn[:], in_=x32s[kt, tt][:])
                    xns[kt, tt] = xn

            wt_cur = unpack_slab(byteT0)

            def emit_xpose(j):
                # bf16 PE transposes (4 token-tiles per PSUM bank); hi
                # (ACT, e4m3 cast out of PSUM) and lo (DVE residual)
                kt, b = divmod(j, 8)
                ps = ps_tr.tile([P, TT, P], BF16, tag="trx_ps")
                for tt in range(TT):
                    nc.tensor.transpose(
                        ps[:, tt, :], xns[kt, tt][:, :, b], ident_bf[:]
                    )
                nc.scalar.copy(out=xT8[:, 2 * j, :], in_=ps[:])
                nc.vector.tensor_tensor(
                    out=xT8[:, 2 * j + 1, :],
                    in0=ps[:].rearrange("p a b -> p (a b)"),
                    in1=xT8[:, 2 * j, :], op=SUB,
                )
                if j == 0:
                    # wraparound lo_0 copy (slot 2NJ is a never-read pad)
                    nc.vector.tensor_copy(
                        out=xT8[:, 2 * NJ + 1, :], in_=xT8[:, 1, :]
                    )

            def pair_ap(j, t0, tn):
                # (hi_j, lo_{j+1}): slots 2j, 2j+3 -> stride-3 stepped slice
                return xT8[:, 2 * j:2 * j + 4:3, t0:t0 + tn]

            s_ps = ps_s_pool.tile([P, T], F32)

            # main o-slab loop; weights for slab sl+1 are emitted before
            # slab sl's matmuls (one-slab prefetch)
            def epilogue(sl, tsub, ps_y, bbc):
                y_sb = y_pool.tile([P, O_SLAB], F32)
                nc.vector.scalar_tensor_tensor(
                    out=y_sb[:], in0=ps_y[:],
                    scalar=neg_s[:, tsub:tsub + 1],
                    in1=bbc[:], op0=ADD, op1=ADD,
                )
                nc.sync.dma_start(
                    y_d[ts(tsub, P), ts(sl, O_SLAB)], y_sb[:]
                )

            for sl in range(NSLAB):
                wt = wt_cur
                pk_next = byte_dma(sl + 1) if sl + 1 < NSLAB else None
                wt8 = wt[:].bitcast(FP8)  # [P, NJ+1, O_SLAB]

                bbc = bbc_pool.tile([P, O_SLAB], F32)
                nc.sync.dma_start(
                    bbc[:],
                    bias_bc[:, ts(sl, O_SLAB)].partition_broadcast(P),
                )
                pss = [
                    ps_mm.tile([P, O_SLAB], F32, name=f"ps{t}", tag="ps_y")
                    for t in range(TT)
                ]
                if sl == 0:
                    # j-major: streams against the hi/lo + unpack
                    # producers; the neg-rowsum MMs ride the same stream
                    for j in range(NJ):
                        emit_xpose(j)
                    for j in range(NJ):
                        nc.tensor.matmul(
                            s_ps[:], nones8[:], pair_ap(j, 0, T),
                            start=(j == 0), stop=(j == NJ - 1),
                            perf_mode=DR,
                        )
                        for tsub in range(TT):
                            nc.tensor.matmul(
                                pss[tsub][:],
                                pair_ap(j, tsub * P, P),
                                wt8[:, j:j + 2, :],
                                start=(j == 0), stop=(j == NJ - 1),
                                perf_mode=DR,
                            )
                    # transpose the [1,T] neg-rowsum back to [t,1] layout
                    nc.vector.tensor_copy(out=srow[:], in_=s_ps[0:1, :])
                    ps_s2 = ps_tr.tile([P, TT, P], F32, tag="trx_ps")
                    for c in range(TT):
                        nc.tensor.transpose(
                            ps_s2[:, c, 0:1], srow[0:1, ts(c, P)],
                            ident_f[0:1, 0:1],
                        )
                    nc.vector.tensor_copy(
                        out=neg_s[:], in_=ps_s2[:, :, 0]
                    )
                    for tsub in range(TT):
                        epilogue(sl, tsub, pss[tsub], bbc)
                    if pk_next is not None:
                        wt_cur = unpack_slab(byte_slab(pk_next))
                else:
                    # t-major: groups finish staggered; each group's
                    # epilogue follows its matmuls so PSUM banks free
                    # early. The next slab's weight compute is emitted
                    # mid-slab so the PE reaches those transposes only
                    # after their inputs are ready (in-order engine).
                    for tsub in range(TT):
                        for j in range(NJ):
                            nc.tensor.matmul(
                                pss[tsub][:],
                                pair_ap(j, tsub * P, P),
                                wt8[:, j:j + 2, :],
                                start=(j == 0), stop=(j == NJ - 1),
                                perf_mode=DR,
                            )
                        epilogue(sl, tsub, pss[tsub], bbc)
                        if tsub == 1 and pk_next is not None:
                            wt_cur = unpack_slab(byte_slab(pk_next))

    nc.compile()
    return nc


_NC = None


def _get_nc():
    global _NC
    if _NC is None:
        _NC = build()
    return _NC


def run(x, packed_weight, bias, trace=False):
    x = np.ascontiguousarray(np.asarray(x, dtype=np.float32))
    pw = np.ascontiguousarray(np.asarray(packed_weight, dtype=np.uint8))
    bias = np.ascontiguousarray(np.asarray(bias, dtype=np.float32))
    assert x.shape == (B_DIM, S_DIM, I_DIM)
    assert pw.shape == (O_DIM, I_DIM // 8)
    assert bias.shape == (O_DIM,)

    nc = _get_nc()
    xs = x.reshape(T_FULL, I_DIM)
    in_maps = [
        {
            "x": np.ascontiguousarray(xs[c * T_SHARD:(c + 1) * T_SHARD]),
            "pw": pw,
            "bias": bias,
        }
        for c in range(N_CORES)
    ]
    res = run_bass_kernel_spmd(nc, in_maps, list(range(N_CORES)), trace=trace)
    y = np.concatenate(
        [res.results[c][OUT_NAME] for c in range(N_CORES)], axis=0
    )
    return y.reshape(B_DIM, S_DIM, O_DIM), res


def kernel(x, packed_weight, bias):
    y, _ = run(x, packed_weight, bias, trace=False)
    return y
